# revision 1
# baseline (speedup 1.0000x reference)
"""Causal self-attention on 8 TRN2 NeuronCores.

Sharding: core c -> (batch b = c//2, head-group g = c%2).
B=4, T=2048, D=1024, 16 heads x 64. Each core computes attention for its
batch and its 8 heads, plus the partial output projection for those heads;
the host sums the two partial projections per batch.

Device layouts (host pre-transposes everything):
  xT    [1024, 2048]  x[b].T
  wqkT  [1024, 1024]  cols 0..511 q-feats, 512..1023 k-feats (group g)
  wvT   [1024, 512]   v-feats (group g)
  wpT   [512, 1024]   w_proj[:, g*512:(g+1)*512].T
  mask  [128, 2048]   4 causal patterns of [128,512] for diag offsets 0/128/256/384
Output: yT [1024, 2048] partial y[b].T (sum over this core's heads).

Attention is computed in S^T orientation (k on partitions, q on free dim):
S^T_j = K_j Q^T via PE, exp on ACT straight out of PSUM, causal masking as a
multiplicative 0/1 mask only on diagonal blocks, and P^T V via PE with an
extra all-ones V column producing the softmax denominators inside the same
accumulation (av row 64). Normalization: DVE reciprocal -> GpSimd
partition-broadcast -> DVE multiply during PSUM evacuation.
All matmuls run in float32r (fp22 multiply, fp32 accumulate, full PE rate).
"""

import sys

for _p in ("/opt/pypackages", "/opt/trn_rl_repo"):
    if _p not in sys.path:
        sys.path.insert(0, _p)

from contextlib import ExitStack

import ml_dtypes
import numpy as np

import concourse.bass as bass
import concourse.tile as tile
from concourse import bacc, mybir
from concourse.bass_utils import run_bass_kernel_spmd

F32 = mybir.dt.float32
F32R = mybir.dt.float32r
BF16 = mybir.dt.bfloat16
AF = mybir.ActivationFunctionType
OP = mybir.AluOpType

D = 1024
T = 2048
NH_LOC = 8          # heads per core
DH = 64
GF = NH_LOC * DH    # 512 features per group

LAST_RESULTS = None
_CACHED = None


def build_program():
    nc = bacc.Bacc("TRN2", target_bir_lowering=False, debug=False)

    xT_d = nc.dram_tensor("xT", [D, T], F32R, kind="ExternalInput").ap()
    wqk_d = nc.dram_tensor("wqkT", [D, 2 * GF], F32R, kind="ExternalInput").ap()
    wv_d = nc.dram_tensor("wvT", [D, GF], F32R, kind="ExternalInput").ap()
    wp_d = nc.dram_tensor("wpT", [GF, D], F32R, kind="ExternalInput").ap()
    mask_d = nc.dram_tensor("mask", [128, 2048], BF16, kind="ExternalInput").ap()
    ones_d = nc.dram_tensor("ones8", [128, 8], BF16, kind="ExternalInput").ap()
    yT_d = nc.dram_tensor("yT", [D, T], F32, kind="ExternalOutput").ap()

    with tile.TileContext(nc) as tc:
        with ExitStack() as octx:
            # ---- persistent pools --------------------------------------
            qk_pool = octx.enter_context(tc.tile_pool(name="qkT", bufs=1))
            v_pool = octx.enter_context(tc.tile_pool(name="vN", bufs=1))
            o_pool = octx.enter_context(tc.tile_pool(name="outT", bufs=1))
            c_pool = octx.enter_context(tc.tile_pool(name="const", bufs=1))

            mask_t = c_pool.tile([128, 2048], BF16, name="mask", tag="mask")
            nc.sync.dma_start(mask_t[:], mask_d[:])

            # qkT: 8 tiles [128,2048]; m 0..3 q-feats, m 4..7 k-feats
            qk_t = [qk_pool.tile([128, T], BF16, name=f"qk{m}", tag=f"qk{m}") for m in range(8)]
            # vN: 16 tiles [128, 520]; cols h*65+0..63 v-feats, col h*65+64 ones
            v_t = [v_pool.tile([128, 8 * (DH + 1)], BF16, name=f"v{t}", tag=f"v{t}") for t in range(16)]
            # outT: 4 tiles [128,2048]; heads (2k,2k+1) -> tile k
            out_t = [o_pool.tile([128, T], F32R, name=f"o{k}", tag=f"o{k}") for k in range(4)]

            # ================= phase 1: qkv projections =================
            with ExitStack() as p1:
                x_pool = p1.enter_context(tc.tile_pool(name="xT", bufs=1))
                w_pool = p1.enter_context(tc.tile_pool(name="wqk", bufs=3))
                wv_pool = p1.enter_context(tc.tile_pool(name="wv", bufs=1))
                ps_qk = p1.enter_context(tc.tile_pool(name="ps_qk", bufs=4, space="PSUM"))
                ps_v = p1.enter_context(tc.tile_pool(name="ps_v", bufs=2, space="PSUM"))

                wv_t = [wv_pool.tile([128, GF], F32R, name=f"wv{k}", tag=f"wv{k}") for k in range(8)]
                for k in range(8):
                    nc.sync.dma_start(wv_t[k][:], wv_d[k * 128:(k + 1) * 128, :])

                for t in range(16):
                    dst = v_t[t][:].rearrange("p (h e) -> p h e", h=8, e=65)[:, :, 64:65]
                    nc.sync.dma_start(dst, ones_d[:].unsqueeze(2))

                # T processed in two halves to halve xT residency
                for half in range(2):
                    t0 = half * 1024
                    x_half = [x_pool.tile([128, 1024], F32R, name=f"x{k}", tag=f"x{k}") for k in range(8)]
                    for k in range(8):
                        nc.sync.dma_start(x_half[k][:], xT_d[k * 128:(k + 1) * 128, t0:t0 + 1024])

                    # q/k features: out[m-feats, t] accumulated over k
                    for m in range(8):
                        pss = [ps_qk.tile([128, 512], F32, name="psqk", tag="psqk") for _ in range(2)]
                        for k in range(8):
                            wt = w_pool.tile([128, 128], F32R, name="w", tag="w")
                            nc.sync.dma_start(
                                wt[:], wqk_d[k * 128:(k + 1) * 128, m * 128:(m + 1) * 128]
                            )
                            for n in range(2):
                                nc.tensor.matmul(
                                    pss[n][:], (wt[:]),
                                    (x_half[k][:, n * 512:(n + 1) * 512]),
                                    start=(k == 0), stop=(k == 7),
                                    skip_group_check=True,
                                )
                        for n in range(2):
                            dst = qk_t[m][:, t0 + n * 512: t0 + (n + 1) * 512]
                            if n == 0:
                                nc.vector.tensor_copy(dst, pss[n][:])
                            else:
                                nc.scalar.activation(dst, pss[n][:], AF.Copy)

                    # v natural: out[t-rows, v-feats] accumulated over k
                    for tt in range(8):
                        psv = ps_v.tile([128, 512], F32, name="psv", tag="psv")
                        for k in range(8):
                            nc.tensor.matmul(
                                psv[:],
                                (x_half[k][:, tt * 128:(tt + 1) * 128]),
                                (wv_t[k][:]),
                                start=(k == 0), stop=(k == 7),
                                skip_group_check=True,
                            )
                        vt = v_t[half * 8 + tt]
                        src = psv[:].rearrange("p (h e) -> p h e", h=8, e=64)
                        dst = vt[:].rearrange("p (h e) -> p h e", h=8, e=65)[:, :, 0:64]
                        nc.vector.tensor_copy(dst, src)

            # ================= phase 2: causal attention ================
            with ExitStack() as p2:
                ps_s = p2.enter_context(tc.tile_pool(name="ps_s", bufs=2, space="PSUM"))
                ps_av = p2.enter_context(tc.tile_pool(name="ps_av", bufs=4, space="PSUM"))
                pt_pool = p2.enter_context(tc.tile_pool(name="pt", bufs=4))
                r_pool = p2.enter_context(tc.tile_pool(name="recip", bufs=4))

                for h in range(NH_LOC):
                    qm = h // 2
                    qoff = 64 * (h % 2)
                    qT = qk_t[qm]
                    kT = qk_t[4 + qm]
                    for c in range(4):          # 512-wide query chunks
                        npieces = 4 * c + 4      # k-blocks 0..npieces-1
                        av = ps_av.tile([65, 512], F32, name="av", tag="av")
                        for w in range(0, npieces, 2):
                            s = ps_s.tile([128, 1024], F32, name="s", tag="s")
                            for idx in range(2):
                                j = w + idx
                                nc.tensor.matmul(
                                    s[:, idx * 512:(idx + 1) * 512],
                                    (kT[qoff:qoff + 64, j * 128:(j + 1) * 128]),
                                    (qT[qoff:qoff + 64, c * 512:(c + 1) * 512]),
                                    start=True, stop=True,
                                    skip_group_check=True,
                                )
                            pt = pt_pool.tile([128, 1024], BF16, name="pt", tag="pt")
                            nc.scalar.activation(pt[:], s[:], AF.Exp, scale=0.125)
                            for idx in range(2):
                                j = w + idx
                                if j // 4 == c:  # diagonal block -> causal mask
                                    d = j * 128 - c * 512
                                    p = d // 128
                                    nc.vector.tensor_tensor(
                                        pt[:, idx * 512:(idx + 1) * 512],
                                        pt[:, idx * 512:(idx + 1) * 512],
                                        mask_t[:, p * 512:(p + 1) * 512],
                                        op=OP.mult,
                                    )
                            for idx in range(2):
                                j = w + idx
                                nc.tensor.matmul(
                                    av[:],
                                    (v_t[j][:, h * 65:(h + 1) * 65]),
                                    (pt[:, idx * 512:(idx + 1) * 512]),
                                    start=(j == 0), stop=(j == npieces - 1),
                                    skip_group_check=True,
                                )
                        # normalize + evacuate
                        den = r_pool.tile([1, 512], F32, name="den", tag="den")
                        nc.vector.tensor_copy(den[:], av[64:65, :])
                        scr = r_pool.tile([1, 512], F32, name="scr", tag="scr")
                        rec = r_pool.tile([1, 512], F32, name="rec", tag="rec")
                        nc.vector.reciprocal_approx_accurate(rec[:], den[:], scratch=scr[:])
                        rb = r_pool.tile([64, 512], F32, name="rb", tag="rb")
                        nc.gpsimd.partition_broadcast(rb[:], rec[:])
                        nc.vector.tensor_tensor(
                            out_t[qm][qoff:qoff + 64, c * 512:(c + 1) * 512],
                            av[0:64, :], rb[:], op=OP.mult,
                        )

            # ================= phase 3: output projection ===============
            with ExitStack() as p3:
                wp_pool = p3.enter_context(tc.tile_pool(name="wp", bufs=1))
                ps_y = p3.enter_context(tc.tile_pool(name="ps_y", bufs=4, space="PSUM"))
                y_pool = p3.enter_context(tc.tile_pool(name="y", bufs=4))

                wp_t = [wp_pool.tile([128, D], F32R, name=f"wp{k}", tag=f"wp{k}") for k in range(4)]
                for k in range(4):
                    nc.sync.dma_start(wp_t[k][:], wp_d[k * 128:(k + 1) * 128, :])

                for m in range(8):
                    for n in range(4):
                        psy = ps_y.tile([128, 512], F32, name="psy", tag="psy")
                        for kk in range(4):
                            nc.tensor.matmul(
                                psy[:],
                                (wp_t[kk][:, m * 128:(m + 1) * 128]),
                                (out_t[kk][:, n * 512:(n + 1) * 512]),
                                start=(kk == 0), stop=(kk == 3),
                                skip_group_check=True,
                            )
                        yt = y_pool.tile([128, 512], F32, name="yst", tag="yst")
                        nc.vector.tensor_copy(yt[:], psy[:])
                        nc.sync.dma_start(
                            yT_d[m * 128:(m + 1) * 128, n * 512:(n + 1) * 512], yt[:]
                        )

    nc.compile()
    return nc


def _make_mask():
    mask = np.zeros((128, 2048), dtype=np.float32)
    kk = np.arange(128)[:, None]
    q = np.arange(512)[None, :]
    for p in range(4):
        d = 128 * p
        mask[:, p * 512:(p + 1) * 512] = ((q - d) >= kk).astype(np.float32)
    return mask


def kernel(x, w_qkv, w_proj):
    global LAST_RESULTS, _CACHED
    x = np.asarray(x, dtype=np.float32)
    w_qkv = np.asarray(w_qkv, dtype=np.float32)
    w_proj = np.asarray(w_proj, dtype=np.float32)
    B = x.shape[0]

    if _CACHED is None:
        _CACHED = build_program()
    nc = _CACHED

    mask = _make_mask()
    in_maps = []
    for c in range(8):
        b, g = c // 2, c % 2
        wq = w_qkv[g * GF:(g + 1) * GF, :]                # [512, 1024]
        wk = w_qkv[D + g * GF: D + (g + 1) * GF, :]
        wv = w_qkv[2 * D + g * GF: 2 * D + (g + 1) * GF, :]
        in_maps.append({
            "xT": np.ascontiguousarray(x[b].T),
            "wqkT": np.ascontiguousarray(np.concatenate([wq, wk], axis=0).T),
            "wvT": np.ascontiguousarray(wv.T),
            "wpT": np.ascontiguousarray(w_proj[:, g * GF:(g + 1) * GF].T),
            "mask": mask.astype(ml_dtypes.bfloat16),
            "ones8": np.ones((128, 8), ml_dtypes.bfloat16),
        })

    res = run_bass_kernel_spmd(nc, in_maps, core_ids=list(range(8)))
    LAST_RESULTS = res

    y = np.empty_like(x)
    for b in range(B):
        yT = res.results[2 * b]["yT"] + res.results[2 * b + 1]["yT"]
        y[b] = yT.T
    return y



# revision 7
# speedup vs baseline: 1.7168x; 1.7168x over previous
"""Causal self-attention on 8 TRN2 NeuronCores.

Sharding: core c -> (batch b = c//2, head-group g = c%2).
B=4, T=2048, D=1024, 16 heads x 64. Each core computes attention for its
batch and its 8 heads, plus the partial output projection for those heads;
the host sums the two partial projections per batch.

v2 design (vs the phase-serial v1):
  * all inputs shipped bf16; weights loaded once as large contiguous tiles
  * heads processed in PAIRS (A on partitions 0-63, B on 64-127); the two
    S^T matmuls of a pair run CONCURRENTLY on the PE via 64x128 row tiling
    (tile_position derives from the operand base partitions)
  * one [128,1024] exp per k-block covers both heads (A|B in adjacent
    PSUM banks)
  * QKV projection of pair p+1 and output-projection chunks are emitted as
    PE filler inside pair p's attention stream, so the PE never idles long
    enough for HAM to re-throttle and the ACT-bound exp stream is hidden
    behind PE work
Device layouts (host pre-transposes; all bf16 except yT):
  xT    [1024, 2048]  x[b].T
  wqkT  [1024, 1024]  cols p*128..  q-feats of head pair p, +512 k-feats
  wvT   [1024, 512]   cols p*128.. v-feats of pair p
  wpT   [512, 1024]   w_proj[:, group].T
  mask  [128, 2048]   4 causal 0/1 patterns of [128,512] for offsets 0..384
Output: yT [1024, 2048] f32 partial y[b].T (sum over this core's heads).
"""

import sys

for _p in ("/opt/pypackages", "/opt/trn_rl_repo"):
    if _p not in sys.path:
        sys.path.insert(0, _p)

from contextlib import ExitStack

import ml_dtypes
import numpy as np

import concourse.bass as bass
import concourse.tile as tile
from concourse import bacc, mybir
from concourse.bass_utils import run_bass_kernel_spmd

F32 = mybir.dt.float32
BF16 = mybir.dt.bfloat16
AF = mybir.ActivationFunctionType
OP = mybir.AluOpType

D = 1024
T = 2048
DH = 64
GF = 512            # features per group (8 heads)
NP = 4              # head pairs per core

LAST_RESULTS = None
_CACHED = None


def build_program():
    nc = bacc.Bacc("TRN2", target_bir_lowering=False, debug=False)

    xT_d = nc.dram_tensor("xT", [D, T], BF16, kind="ExternalInput").ap()
    wqk_d = nc.dram_tensor("wqkT", [D, 2 * GF], BF16, kind="ExternalInput").ap()
    wv_d = nc.dram_tensor("wvT", [D, GF], BF16, kind="ExternalInput").ap()
    wp_d = nc.dram_tensor("wpT", [GF, D], BF16, kind="ExternalInput").ap()
    mask_d = nc.dram_tensor("mask", [128, 2048], BF16, kind="ExternalInput").ap()
    yT_d = nc.dram_tensor("yT", [D, T], F32, kind="ExternalOutput").ap()

    with tile.TileContext(nc) as tc:
        with ExitStack() as octx:
            # ---- persistent pools --------------------------------------
            c_pool = octx.enter_context(tc.tile_pool(name="const", bufs=1))
            x_pool = octx.enter_context(tc.tile_pool(name="xT", bufs=1))
            wqk_pool = octx.enter_context(tc.tile_pool(name="wqk", bufs=1))
            wv_pool = octx.enter_context(tc.tile_pool(name="wv", bufs=1))
            wp_pool = octx.enter_context(tc.tile_pool(name="wp", bufs=1))
            qk_pool = octx.enter_context(tc.tile_pool(name="qkT", bufs=1))
            v_pool = octx.enter_context(tc.tile_pool(name="vN", bufs=1))
            o_pool = octx.enter_context(tc.tile_pool(name="outT", bufs=1))
            pt_pool = octx.enter_context(tc.tile_pool(name="pt", bufs=3))
            r_pool = octx.enter_context(tc.tile_pool(name="recip", bufs=4))
            y_pool = octx.enter_context(tc.tile_pool(name="y", bufs=4))
            ps_s = octx.enter_context(tc.tile_pool(name="ps_s", bufs=2, space="PSUM"))
            ps_av = octx.enter_context(tc.tile_pool(name="ps_av", bufs=1, space="PSUM"))
            ps_f = octx.enter_context(tc.tile_pool(name="ps_f", bufs=2, space="PSUM"))

            mask_t = c_pool.tile([128, 2048], BF16, name="mask", tag="mask")
            nc.sync.dma_start(mask_t[:], mask_d[:])

            x_t = [x_pool.tile([128, T], BF16, name=f"x{k}", tag=f"x{k}") for k in range(8)]
            for k in range(8):
                nc.sync.dma_start(x_t[k][:], xT_d[k * 128:(k + 1) * 128, :])
            wqk_t = [wqk_pool.tile([128, 2 * GF], BF16, name=f"wqk{k}", tag=f"wqk{k}") for k in range(8)]
            for k in range(8):
                nc.sync.dma_start(wqk_t[k][:], wqk_d[k * 128:(k + 1) * 128, :])
            wv_t = [wv_pool.tile([128, GF], BF16, name=f"wv{k}", tag=f"wv{k}") for k in range(8)]
            for k in range(8):
                nc.sync.dma_start(wv_t[k][:], wv_d[k * 128:(k + 1) * 128, :])
            wp_t = [wp_pool.tile([128, D], BF16, name=f"wp{k}", tag=f"wp{k}") for k in range(4)]
            for k in range(4):
                nc.sync.dma_start(wp_t[k][:], wp_d[k * 128:(k + 1) * 128, :])

            # qkT: per pair p, tile 2p = qT, 2p+1 = kT; rows 0-63 head A
            # features, 64-127 head B
            qk_t = [qk_pool.tile([128, T], BF16, name=f"qk{m}", tag=f"qk{m}") for m in range(8)]
            # vN: 16 tiles [128 t, 520]; cols h*65+0..63 v-feats, h*65+64 ones
            v_t = [v_pool.tile([128, 8 * (DH + 1)], BF16, name=f"v{t}", tag=f"v{t}") for t in range(16)]
            for t in range(16):
                dst = v_t[t][:].rearrange("p (h e) -> p h e", h=8, e=65)[:, :, 64:65]
                nc.vector.memset(dst, 1.0)
            # outT: per pair p [128, 2048]; rows 0-63 head A out, 64-127 B
            out_t = [o_pool.tile([128, T], BF16, name=f"o{k}", tag=f"o{k}") for k in range(4)]

            # ---- HAM warmup: keep the PE busy while input DMAs land ----
            wm = ps_f.tile([128, 512], F32, name="warm", tag="psf")
            for _ in range(18):
                nc.tensor.matmul(
                    wm[:], (mask_t[0:128, 0:128]), (mask_t[:, 0:512]),
                    start=True, stop=True, skip_group_check=True,
                )

            # ---- filler generators (QKV projection / output projection) --
            def emit_qk_unit(p, which, n):
                # q (which=0) / k (which=1) features of pair p, t-chunk n
                ps = ps_f.tile([128, 512], F32, name="psqk", tag="psf")
                col = which * 512 + p * 128
                for k in range(8):
                    nc.tensor.matmul(
                        ps[:], (wqk_t[k][:, col:col + 128]),
                        (x_t[k][:, n * 512:(n + 1) * 512]),
                        start=(k == 0), stop=(k == 7), skip_group_check=True,
                    )
                nc.vector.tensor_copy(qk_t[2 * p + which][:, n * 512:(n + 1) * 512], ps[:])

            def emit_v_unit(p, t):
                # v features of pair p for t-tile t (natural orientation)
                ps = ps_f.tile([128, 512], F32, name="psv", tag="psf")
                for k in range(8):
                    nc.tensor.matmul(
                        ps[:, 0:128], (x_t[k][:, t * 128:(t + 1) * 128]),
                        (wv_t[k][:, p * 128:(p + 1) * 128]),
                        start=(k == 0), stop=(k == 7), skip_group_check=True,
                    )
                src = ps[:, 0:128].rearrange("p (h e) -> p h e", h=2, e=64)
                dst = v_t[t][:].rearrange("p (h e) -> p h e", h=8, e=65)[:, 2 * p:2 * p + 2, 0:64]
                nc.vector.tensor_copy(dst, src)

            def qkv_units(p):
                for which in range(2):
                    for n in range(4):
                        yield ('qk', p, which, n)
                for t in range(16):
                    yield ('v', p, t)

            def emit_proj_unit(m, n):
                # y[m-feats, t-chunk n] = sum_p wp_p^T @ out_p
                ps = ps_f.tile([128, 512], F32, name="psy", tag="psf")
                for kk in range(4):
                    nc.tensor.matmul(
                        ps[:], (wp_t[kk][:, m * 128:(m + 1) * 128]),
                        (out_t[kk][:, n * 512:(n + 1) * 512]),
                        start=(kk == 0), stop=(kk == 3), skip_group_check=True,
                    )
                yt = y_pool.tile([128, 512], F32, name="yst", tag="yst")
                nc.vector.tensor_copy(yt[:], ps[:])
                nc.sync.dma_start(
                    yT_d[m * 128:(m + 1) * 128, n * 512:(n + 1) * 512], yt[:]
                )

            def emit_unit(u):
                if u[0] == 'qk':
                    emit_qk_unit(u[1], u[2], u[3])
                elif u[0] == 'v':
                    emit_v_unit(u[1], u[2])
                else:
                    emit_proj_unit(u[1], u[2])

            # pair 0 QKV upfront (dense)
            for u in qkv_units(0):
                emit_unit(u)

            # ---- attention over head pairs, QKV/proj interleaved -------
            for p in range(NP):
                qT = qk_t[2 * p]
                kT = qk_t[2 * p + 1]
                sA = 2 * p * 65          # v_t column slots
                sB = (2 * p + 1) * 65
                if p < NP - 1:
                    filler = list(qkv_units(p + 1))
                else:
                    filler = [('proj', m, n) for n in range(3) for m in range(8)]
                # slots where filler units may be emitted: after each j pair
                fi = 0

                for c in range(4):
                    npieces = 4 * c + 4
                    avA = ps_av.tile([65, 512], F32, name="avA", tag="avA")
                    avB = ps_av.tile([65, 512], F32, name="avB", tag="avB")
                    for w in range(0, npieces, 2):
                        ss = []
                        for idx in range(2):
                            j = w + idx
                            s = ps_s.tile([128, 1024], F32, name="s", tag="s")
                            nc.tensor.matmul(
                                s[:, 0:512],
                                (kT[0:64, j * 128:(j + 1) * 128]),
                                (qT[0:64, c * 512:(c + 1) * 512]),
                                start=True, stop=True, skip_group_check=True,
                            )
                            nc.tensor.matmul(
                                s[:, 512:1024],
                                (kT[64:128, j * 128:(j + 1) * 128]),
                                (qT[64:128, c * 512:(c + 1) * 512]),
                                start=True, stop=True, skip_group_check=True,
                            )
                            ss.append(s)
                        pts = []
                        for idx in range(2):
                            j = w + idx
                            pt = pt_pool.tile([128, 1024], BF16, name="pt", tag="pt")
                            nc.scalar.activation(pt[:], ss[idx][:], AF.Exp, scale=0.125)
                            if j // 4 == c:  # diagonal block -> causal mask
                                mp = (j * 128 - c * 512) // 128
                                for half in range(2):
                                    nc.vector.tensor_tensor(
                                        pt[:, half * 512:(half + 1) * 512],
                                        pt[:, half * 512:(half + 1) * 512],
                                        mask_t[:, mp * 512:(mp + 1) * 512],
                                        op=OP.mult,
                                    )
                            pts.append(pt)
                        for idx in range(2):
                            j = w + idx
                            nc.tensor.matmul(
                                avA[:], (v_t[j][:, sA:sA + 65]),
                                (pts[idx][:, 0:512]),
                                start=(j == 0), stop=(j == npieces - 1),
                                skip_group_check=True,
                            )
                            nc.tensor.matmul(
                                avB[:], (v_t[j][:, sB:sB + 65]),
                                (pts[idx][:, 512:1024]),
                                start=(j == 0), stop=(j == npieces - 1),
                                skip_group_check=True,
                            )
                        # one filler unit per j-pair keeps the PE queue fed
                        # while ACT works through the exps
                        if fi < len(filler):
                            # pair-3 proj filler for t-chunk n must wait for
                            # out_t[3][:, n] -> only emit n < c units
                            u = filler[fi]
                            if p < NP - 1 or u[2] < c:
                                emit_unit(u)
                                fi += 1
                    # ---- normalize + evacuate this q-chunk ----
                    for av, row0 in ((avA, 0), (avB, 64)):
                        den = r_pool.tile([1, 512], F32, name="den", tag="den")
                        nc.vector.tensor_copy(den[:], av[64:65, :])
                        scr = r_pool.tile([1, 512], F32, name="scr", tag="scr")
                        rec = r_pool.tile([1, 512], F32, name="rec", tag="rec")
                        nc.vector.reciprocal_approx_accurate(rec[:], den[:], scratch=scr[:])
                        rb = r_pool.tile([64, 512], F32, name="rb", tag="rb")
                        nc.gpsimd.partition_broadcast(rb[:], rec[:])
                        nc.vector.tensor_tensor(
                            out_t[p][row0:row0 + 64, c * 512:(c + 1) * 512],
                            av[0:64, :], rb[:], op=OP.mult,
                        )
                # flush any filler not emitted during the attention loop
                while fi < len(filler):
                    emit_unit(filler[fi])
                    fi += 1

            # ---- projection tail (t-chunk 3) ---------------------------
            for m in range(8):
                emit_proj_unit(m, 3)

    nc.compile()
    return nc


def _make_mask():
    mask = np.zeros((128, 2048), dtype=np.float32)
    kk = np.arange(128)[:, None]
    q = np.arange(512)[None, :]
    for p in range(4):
        d = 128 * p
        mask[:, p * 512:(p + 1) * 512] = ((q - d) >= kk).astype(np.float32)
    return mask


def kernel(x, w_qkv, w_proj):
    global LAST_RESULTS, _CACHED
    x = np.asarray(x, dtype=np.float32)
    w_qkv = np.asarray(w_qkv, dtype=np.float32)
    w_proj = np.asarray(w_proj, dtype=np.float32)
    B = x.shape[0]

    if _CACHED is None:
        _CACHED = build_program()
    nc = _CACHED

    mask = _make_mask()
    in_maps = []
    for c in range(8):
        b, g = c // 2, c % 2
        wq = w_qkv[g * GF:(g + 1) * GF, :]                # [512, 1024]
        wk = w_qkv[D + g * GF: D + (g + 1) * GF, :]
        wv = w_qkv[2 * D + g * GF: 2 * D + (g + 1) * GF, :]
        in_maps.append({
            "xT": np.ascontiguousarray(x[b].T).astype(ml_dtypes.bfloat16),
            "wqkT": np.ascontiguousarray(np.concatenate([wq, wk], axis=0).T).astype(ml_dtypes.bfloat16),
            "wvT": np.ascontiguousarray(wv.T).astype(ml_dtypes.bfloat16),
            "wpT": np.ascontiguousarray(w_proj[:, g * GF:(g + 1) * GF].T).astype(ml_dtypes.bfloat16),
            "mask": mask.astype(ml_dtypes.bfloat16),
        })

    res = run_bass_kernel_spmd(nc, in_maps, core_ids=list(range(8)))
    LAST_RESULTS = res

    y = np.empty_like(x)
    for b in range(B):
        yT = res.results[2 * b]["yT"] + res.results[2 * b + 1]["yT"]
        y[b] = yT.T
    return y
